# revision 95
# baseline (speedup 1.0000x reference)
"""Trainium2 Bass kernel for GQA causal sliding-window self-attention.

Problem: B=2, T=2048, C=1024, 16 heads (hd=64), 4 KV groups, window=256.
  q = x@Wq+bq; k = x@Wk+bk; v = x@Wv+bv  (GQA repeat of kv over 4 heads)
  att = softmax(mask(q k^T / 8));  y = (att v) @ Wo + bo

Sharding: data-parallel over (batch, T-chunk). 8 cores = 2 batches x 4
chunks of 512 query tokens. Each core receives the 768-token extended
x slice (512 queries + 256 halo for the window) and computes its chunk's
output rows completely locally; no collectives.

Per-core design (bf16 + fp8 q/k projections, 3-ktile band attention):
  The q/k projections run as fp8e4 DoubleRow matmuls (0.5 cycles/row,
  2x128 contraction pairs over kc): x is quantized to fp8 for the q/k
  path only (the v and output paths keep bf16 x, whose error feeds the
  output directly), and Wq/Wk are split hi+lo fp8 at scale 64 so their
  quantization error is negligible; the 64^2*sqrt(hd) descale folds into
  the Exp activation's scale operand for free. Measured end-to-end rel
  err 8.0e-3 (vs 3.1e-3 all-bf16, 2e-2 gate) for ~40% less q/k-proj PE
  time. Everything else is bf16 (PSUM accumulation stays fp32).
  Attention is tiled as 128-query tiles; each q-tile only touches
  its 3 surrounding 128-token k-tiles (384 k vs the naive 512). Only the
  two outer k-tiles carry band masks — two constant 128x128 triangles
  multiplied into exp(scores), split across DVE and Pool; the middle
  tile is all-valid, and left-halo tokens are excluded via a per-core
  0/1 validity vector that becomes the v ones-column (halo v rows are
  zero, so both numerator and denominator drop them exactly).
  Head pairs share a 2-bank score psum tile [128, 2, 512]; one Exp
  activation per pair writes bf16 att to SBUF. att@v runs token-major:
  lhsT = att tile [128k, 128q], rhs = v (token-major, ones column
  appended) -> y psum [128q, 2, 65] per pair whose column 64 is the
  softmax denominator — per-PARTITION, so the reciprocal is a cheap
  [128, 2] DVE op and normalization fuses into the PSUM->SBUF copies
  (tensor_scalar_mul). The normalized token-major y [128q, 128feat] per
  head pair is transposed back to feature-major with a PE transpose
  through a DMA'd identity (128 cycles) + a DVE copy into per-(m, qt)
  yn tiles that the output projection consumes as lhsT.
  Biases fold into the projections as rank-1 ones-row accumulate
  matmuls (bk) or DVE tensor_scalar adds (bq, 1/8-scaled host-side).

Schedule: a software-pipelined flat loop over the 64 (head, qtile)
units — scores+exp+mask at step s, att@v at lag AV_LAG, the
normalize/reciprocal tail at TAIL1_LAG, and the transpose+copy at
TAIL2_LAG — so no engine's in-order queue ever blocks on a younger
dependency. q projections interleave with qt-0 units; each token
tile's output projection is emitted at the first step where all its
yn writes exist (races verified gone with CoreSim's race detector).
Few big HWDGE DMAs ordered by consumption deadline.

Host folds 1/sqrt(64) into Wq/bq and applies the exactly-linear bv/bo
corrections after the device pass:  out += bv_rep @ Wo + bo.

Environment workarounds (this container's walrus build): max 1 sync wait
per CTRL-class instruction (see _split_multi_waits and the chunked tail
drain).
"""
import sys

sys.path.insert(0, "/opt/trn_rl_repo")

import contextlib

import numpy as np
import ml_dtypes

import concourse.bass as bass
import concourse.tile as tile
from concourse import mybir
from concourse.bass_utils import run_bass_kernel_spmd
from concourse.vector_clock import ScopedClock

F32 = mybir.dt.float32
BF16 = mybir.dt.bfloat16
F8 = mybir.dt.float8e4
NPBF16 = ml_dtypes.bfloat16
NPF8 = ml_dtypes.float8_e4m3

B, T, C = 2, 2048, 1024
NH, NG, HD = 16, 4, 64
KV = NG * HD  # 256
WINDOW = 256
NCORES = 8
TQ = 512  # query tokens per core
TE = TQ + WINDOW  # 768 extended tokens per core
KC = C // 128  # 8 contraction tiles
NQT = TQ // 128  # 4 query tiles per core
AV_LAG, TAIL1_LAG, TAIL2_LAG = 7, 11, 15
DEBUG = False
MASK_ENG = lambda nc: nc.gpsimd
PAIRED_EXP = True
TP_POOL_NAME = "pj"
TP_POOL = [None, None]
OP_M, OP_GAP, OP_SLACK = 4, 3, 0  # out-proj insert step tuning  # software-pipeline stage lags (in units)
DMA_TP_QT = 0  # pairs with qt < this use DMA XBAR transpose (PE+DVE otherwise)
XT_FIRST = False  # issue the xt half DMA before wk
FIN_FAST = False  # split mid-stream out_proj_fin copies scalar/DVE + 2 DMAs
FIN_FAST_LAST = False  # same for the final two fins
LAST_DMA_SCALAR = False  # final n2=1 store issued from the Act hwdge queue
FIN_ENG = "scalar"  # engine for the mid-stream (tt<3) out-proj fin copies
OUT_BF16 = True  # store the output as bf16 (halves the final DMA bytes)
FIN2_DVE = False  # final n2=1 fin copy on DVE (measured neutral; Act default)
AV_MID_FIRST = True  # av accumulates the unmasked middle k-tile first
NORM_DIV = False  # tail1 fused divide (measured +561ns vs recip+mult)
OP2B_OFF = 5  # drain: out_proj_half(2,1) at idx n-TAIL2_LAG+OP2B_OFF
OP3_OFF = 3  # drain: tt=3 pre-accumulation starts at idx n-OP3_OFF
Q_STEP = 2  # q_proj(m) inserted at unit step m*Q_STEP
SC_BUFS, PJ_BUFS, YP_BUFS = 2, 2, 2  # PSUM pool rotation depths
MERGED = False  # merged-head score matmuls: 3 per pair (free 256) vs 6 (free 128)
QT_HH1_ENG = "vector"  # engine for the shifted qT repack half (merged only)
YN_ENG = "vector"  # engine for the tp->yn copy-back
# fp8e4 DoubleRow q/k projections: x quantized to fp8 (v/out paths keep bf16
# x), W split hi+lo fp8 so weight-quant error is negligible. Measured end-to-
# end rel err 8.5e-3 vs 3.1e-3 all-bf16 (threshold 2e-2). Halves q/k-proj PE.
FP8QK = True
WS = 64.0  # fp8 weight scale (Wq,Wk,bq,bk carry x64; undone in the exp scale)
# fp8 DoubleRow score matmuls (needs FP8QK): qT/kT written as fp8 at scale 16
# (the x0.25 rescale keeps values away from fp8 saturation), stationary pair
# lane 1 is zeros, moving pair broadcast — 0.5 cycles/column.
SCORES_FP8 = True
EXPSCALE = (
    1.0 / (16.0 * 16.0 * 8.0) if SCORES_FP8 else 1.0 / (WS * WS * 8.0)
)  # scores carry (qk scale)^2; 1/8 = 1/sqrt(hd)
NORM_BCAST = True  # tail1 normalize as one broadcast tensor_tensor
DEFER_S2 = False  # interleave the s2=1 k/v projections into early unit steps
DEFER_K = False  # also defer the s2=1 k halves (risks kT false deps)
KV2_STEPS = (1, 3, 5, 7, 9)  # insert steps for k_half(1,0), k_half(1,1), v3..v5


class _ChunkedDrainTileContext(tile.TileContext):
    """Walrus in this container only accepts 1 sync wait on CTRL-class
    instructions; the stock Tile tail drain carries one wait per
    outstanding proc. Spread them over SP nops first, and use the cheaper
    sem-only barriers for the tail."""

    def _drain_and_barrier(self, tick_clock, wait_clock):
        gc = tick_clock.global_clock
        entries = []
        for scope, vc in ScopedClock({None: gc}).items():
            for proc in range(len(vc)):
                t = vc[proc]
                if t > 0:
                    entries.append((scope, proc, t))
        # Spread the one-wait-per-instruction tail waits across engines so
        # they resolve in parallel; the sem-only barrier then syncs engines.
        engines = [self.nc.sync, self.nc.vector, self.nc.scalar, self.nc.gpsimd]
        curs = [ScopedClock() for _ in engines]
        for i, (scope, proc, t) in enumerate(entries):
            eng = engines[i % len(engines)]
            nop = eng.nop(nofuse=True, hint="tail_wait")
            partial = ScopedClock()
            partial.require_at_least(scope, proc, t)
            wait_clock.add_sem_waits(nop.ins, partial, curs[i % len(engines)])
            curs[i % len(engines)].update_past(partial)
        self.nc.all_engine_barrier(sem_only=True)
        drain_inst = self.nc.sync.drain()
        cur = ScopedClock()
        for c in curs:
            cur.update_past(c)
        wait_clock.add_sem_waits(drain_inst.ins, ScopedClock({None: gc}), cur)
        assert self.sems is not None
        popped = self.nc._tile_sem_poison_stack.pop()
        assert popped is self._sem_poison
        self.nc.clear_and_free_semaphores(list(self.sems.allocated().values()))


def _split_multi_waits(nc, max_waits=1):
    """This walrus build rejects >1 sync wait on several instruction structs
    (CTRL, self-loading fp32r Matmult). Hoist excess waits onto same-engine
    NOPs placed immediately before the instruction — identical semantics."""
    fn = nc.m.functions[0]
    for blk in fn.blocks:
        insts = blk.instructions
        new = []
        changed = False
        for inst in insts:
            si = inst.sync_info
            waits = list(si.on_wait) if si is not None and si.on_wait else []
            if len(waits) > max_waits:
                changed = True
                for w in waits[:-max_waits]:
                    nop = mybir.InstNoOp(
                        name=nc.get_next_instruction_name(),
                        ins=[],
                        outs=[],
                        engine=inst.engine,
                        sync_info=mybir.SyncInfo(on_wait=[w], on_update=[]),
                        bass_nofuse=True,
                    )
                    nc.register_instruction(nop, overwrite=True)
                    new.append(nop)
                si.on_wait = waits[-max_waits:]
                inst.sync_info = si
            new.append(inst)
        if changed:
            blk.instructions = new


MM_LABELS = []


def _build_program():
    MM_LABELS.clear()
    nc = bass.Bass("TRN2", target_bir_lowering=False, debug=False, num_devices=NCORES)

    xt = nc.dram_tensor("xt", [128, KC, TE], BF16, kind="ExternalInput")
    if FP8QK:
        # fp8 operands for the q/k projections (DoubleRow pairs over kc)
        xt8 = nc.dram_tensor("xt8", [128, 4, 2, TE], F8, kind="ExternalInput")
        wq = nc.dram_tensor("wq", [4, 128, 2, 2, 4, 2, 128], F8, kind="ExternalInput")
        wk = nc.dram_tensor("wk", [128, 2, 2, 4, 2, 128], F8, kind="ExternalInput")
    else:
        wq = nc.dram_tensor("wq", [KC, 128, KC, 128], BF16, kind="ExternalInput")
        wk = nc.dram_tensor("wk", [2, 128, KC, 128], BF16, kind="ExternalInput")
    wv = nc.dram_tensor("wv", [128, KC, KV], BF16, kind="ExternalInput")
    wo = nc.dram_tensor("wo", [128, KC, C], BF16, kind="ExternalInput")
    bq = nc.dram_tensor("bq", [64, 2, KC] if MERGED else [C, 1], F32, kind="ExternalInput")
    bk = nc.dram_tensor("bk", [1, KV], BF16, kind="ExternalInput")
    maskp = nc.dram_tensor("maskp", [128, 2, 128], BF16, kind="ExternalInput")
    vcol = nc.dram_tensor("vcol", [128, 6, NG], BF16, kind="ExternalInput")
    ident = nc.dram_tensor("ident", [128, 128], BF16, kind="ExternalInput")
    out = nc.dram_tensor("out", [TQ, C], BF16 if OUT_BF16 else F32, kind="ExternalOutput")
    dbg = {}
    if DEBUG:
        dbg['kT0'] = nc.dram_tensor("d_kT0", [128, TE], F32, kind="ExternalOutput")
        dbg['v2'] = nc.dram_tensor("d_v2", [128, NG * (HD + 1)], F32, kind="ExternalOutput")
        dbg['qT0'] = nc.dram_tensor("d_qT0", [128, TQ], F32, kind="ExternalOutput")
        dbg['ex00'] = nc.dram_tensor("d_ex00", [128, 2, 3 * 128], F32, kind="ExternalOutput")
        dbg['y00'] = nc.dram_tensor("d_y00", [128, 2 * (HD + 1)], F32, kind="ExternalOutput")
        dbg['ytn00'] = nc.dram_tensor("d_ytn00", [128, 2 * HD], F32, kind="ExternalOutput")
        dbg['yn00'] = nc.dram_tensor("d_yn00", [128, 128], F32, kind="ExternalOutput")

    with _ChunkedDrainTileContext(nc) as tc:
        with contextlib.ExitStack() as ctx:
            wsb = ctx.enter_context(tc.tile_pool(name="wsb", bufs=1))
            xsb = ctx.enter_context(tc.tile_pool(name="xsb", bufs=1))
            csb = ctx.enter_context(tc.tile_pool(name="csb", bufs=1))
            qkv = ctx.enter_context(tc.tile_pool(name="qkv", bufs=1))
            ynp = ctx.enter_context(tc.tile_pool(name="ynp", bufs=1))
            expp = ctx.enter_context(tc.tile_pool(name="expp", bufs=8))
            rrp = ctx.enter_context(tc.tile_pool(name="rrp", bufs=8))
            ytnp = ctx.enter_context(tc.tile_pool(name="ytnp", bufs=8))
            outp = ctx.enter_context(tc.tile_pool(name="outp", bufs=2))
            pj = ctx.enter_context(tc.tile_pool(name="pj", bufs=PJ_BUFS, space="PSUM"))
            scp_pool = ctx.enter_context(tc.tile_pool(name="scp", bufs=SC_BUFS, space="PSUM"))
            yp_pool = ctx.enter_context(tc.tile_pool(name="yp", bufs=YP_BUFS, space="PSUM"))

            TP_POOL[0] = {"pj": pj, "yp": yp_pool, "sc": scp_pool}[TP_POOL_NAME]
            TP_POOL[1] = {"pj": "pj", "yp": "y4", "sc": "sc"}[TP_POOL_NAME]

            # ---- loads (few big HWDGE DMAs, ordered by consumption deadline)
            xt_all = xsb.tile([128, KC, TE], BF16, name="xt_all", tag="xt_all")
            if FP8QK:
                wk_all = wsb.tile([128, 2, 2, 4, 2, 128], F8, name="wk_all", tag="wk_all")
                xt8_sb = xsb.tile([128, 4, 2, TE], F8, name="xt8", tag="xt8")
                nc.sync.dma_start(out=wk_all[:], in_=wk[:])
                # one DMA: token-halved transfers would pay the <512B-
                # descriptor 2x latency penalty (384B runs); a kcp split
                # costs an extra HWDGE gen for no net gain (measured)
                nc.sync.dma_start(out=xt8_sb[:], in_=xt8[:])
                nc.sync.dma_start(out=xt_all[:, 0:4, 0:384], in_=xt[:, 0:4, 0:384])
                nc.sync.dma_start(out=xt_all[:, 4:KC, 0:384], in_=xt[:, 4:KC, 0:384])
            else:
                wk_all = wsb.tile([128, 2, KC, 128], BF16, name="wk_all", tag="wk_all")
                if XT_FIRST:
                    nc.sync.dma_start(out=xt_all[:, 0:4, 0:384], in_=xt[:, 0:4, 0:384])
                    nc.sync.dma_start(out=wk_all[:, 0, 0:4], in_=wk[0, :, 0:4])
                else:
                    nc.sync.dma_start(out=wk_all[:, 0, 0:4], in_=wk[0, :, 0:4])
                    nc.sync.dma_start(out=xt_all[:, 0:4, 0:384], in_=xt[:, 0:4, 0:384])
                nc.sync.dma_start(out=wk_all[:, 0, 4:KC], in_=wk[0, :, 4:KC])
                nc.sync.dma_start(out=xt_all[:, 4:KC, 0:384], in_=xt[:, 4:KC, 0:384])
                nc.sync.dma_start(out=wk_all[:, 1], in_=wk[1])
            bk_row = csb.tile([1, KV], BF16)
            vcol_sb = csb.tile([128, 6, NG], BF16)
            wv_all = wsb.tile([128, KC, KV], BF16, name="wv_all", tag="wv_all")
            bq_all = csb.tile([64, 2, KC] if MERGED else [128, KC], F32)
            mask_sb = csb.tile([128, 2, 128], BF16)
            id_sb = csb.tile([128, 128], BF16)
            wo_all = wsb.tile([128, KC, C], BF16, name="wo_all", tag="wo_all")
            ones_row = csb.tile([1, 384], BF16)
            nc.vector.memset(ones_row[:], 1.0)
            wq_sb = [None] * 4

            def _load_wq(mp):
                if FP8QK:
                    t = wsb.tile([128, 2, 2, 4, 2, 128], F8, name=f"wq{mp}", tag=f"wq{mp}")
                    nc.sync.dma_start(out=t[:], in_=wq[mp])
                else:
                    t = wsb.tile([128, 2, KC, 128], BF16, name=f"wq{mp}", tag=f"wq{mp}")
                    nc.sync.dma_start(
                        out=t[:],
                        in_=wq[2 * mp : 2 * mp + 2].rearrange("m p k b -> p m k b"),
                    )
                wq_sb[mp] = t

            # remaining loads by consumption deadline: bk/vcol for the
            # prologue copies, wv for v-proj, wq0+bq for the first q-proj,
            # xt upper half for the s2=1 prologue, masks for the first pair,
            # ident only for the tail transposes
            nc.sync.dma_start(out=bk_row[:], in_=bk[:])
            nc.sync.dma_start(out=vcol_sb[:], in_=vcol[:])
            nc.sync.dma_start(out=wv_all[:], in_=wv[:])
            _load_wq(0)
            if MERGED:
                nc.sync.dma_start(out=bq_all[:], in_=bq[:])
            else:
                nc.sync.dma_start(out=bq_all[:], in_=bq[:, 0].rearrange("(m p) -> p m", p=128))
            nc.sync.dma_start(out=xt_all[:, :, 384:TE], in_=xt[:, :, 384:TE])
            nc.sync.dma_start(out=mask_sb[:], in_=maskp[:])
            _load_wq(1)
            nc.sync.dma_start(out=id_sb[:], in_=ident[:])
            for mp in range(2, 4):
                _load_wq(mp)
            nc.sync.dma_start(out=wo_all[:], in_=wo[:])

            # ---- kT projection ----
            # MERGED: kT[g] = [64 hd, te]; else duplicated halves [128, te].
            if SCORES_FP8:
                # fp8 kT with an interleaved zero pair-lane: DoubleRow scores
                # compute kT.T@qT + 0.T@junk at 0.5 cycles/column. The zero
                # lane is memset once on gpsimd (idle during the prologue).
                kT_sb = [
                    qkv.tile([128, 2, TE], F8, name=f"kT{g}", tag=f"kT{g}")
                    for g in range(NG)
                ]
                for g in range(NG):
                    nc.gpsimd.memset(kT_sb[g][:, 1, :], 0.0)
            else:
                kT_sb = [
                    qkv.tile([64 if MERGED else 128, TE], BF16, name=f"kT{g}", tag=f"kT{g}")
                    for g in range(NG)
                ]
            v_sb = [None] * 6

            def _dump(name, ap):
                if DEBUG and name in dbg:
                    st = csb.tile(list(dbg[name].shape), F32, name=f"dbg_{name}", tag=f"dbg_{name}")
                    nc.vector.tensor_copy(st[:], ap)
                    nc.sync.dma_start(out=dbg[name][:], in_=st[:])
            def k_half(s2, mt):
                kp = pj.tile([128, 512], F32, name="kp", tag="pj")
                if FP8QK:
                    for kcp in range(4):
                        for hl in range(2):
                            nc.tensor.matmul(
                                kp[:, 0:384],
                                wk_all[:, mt, hl, kcp, :, :],
                                xt8_sb[:, kcp, :, s2 * 384 : (s2 + 1) * 384],
                                start=(hl == 0 and kcp == 0),
                                stop=False,
                                perf_mode=mybir.MatmulPerfMode.DoubleRow,
                            )
                else:
                    for kc in range(KC):
                        nc.tensor.matmul(
                            kp[:, 0:384],
                            wk_all[:, mt, kc, :],
                            xt_all[:, kc, s2 * 384 : (s2 + 1) * 384],
                            start=(kc == 0),
                            stop=False,
                        )
                # bk folded in as a rank-1 accumulate: += bk^T . ones
                nc.tensor.matmul(
                    kp[:, 0:384],
                    bk_row[:, mt * 128 : (mt + 1) * 128],
                    ones_row[:],
                    start=False,
                    stop=True,
                )
                for gh in range(2):  # source half (group g = 2*mt+gh)
                    g = 2 * mt + gh
                    if MERGED:
                        eng = nc.scalar if gh % 2 == 0 else nc.vector
                        (eng.copy if eng is nc.scalar else eng.tensor_copy)(
                            kT_sb[g][0:64, s2 * 384 : (s2 + 1) * 384],
                            kp[gh * 64 : gh * 64 + 64, 0:384],
                        )
                        continue
                    for half in range(2):  # dest duplicated half
                        eng = nc.scalar if (gh + half) % 2 == 0 else nc.vector
                        if SCORES_FP8:
                            # x0.25 rescale (64->16) keeps fp8 away from the
                            # +-240 saturation point; fp8 write, zero lane kept
                            dst = kT_sb[g][
                                half * 64 : half * 64 + 64, 0, s2 * 384 : (s2 + 1) * 384
                            ]
                            src = kp[gh * 64 : gh * 64 + 64, 0:384]
                            if eng is nc.scalar:
                                eng.mul(dst, src, 0.25)
                            else:
                                eng.tensor_scalar_mul(dst, src, 0.25)
                            continue
                        (eng.copy if eng is nc.scalar else eng.tensor_copy)(
                            kT_sb[g][half * 64 : half * 64 + 64, s2 * 384 : (s2 + 1) * 384],
                            kp[gh * 64 : gh * 64 + 64, 0:384],
                        )

            def v_half(vt, deferred=False):
                # v projection (token-major, ones column) for tile vt;
                # deferred halves borrow pj so they don't displace score tiles
                if deferred:
                    vp = pj.tile([128, 512], F32, name="vp", tag="pj")
                else:
                    vp = scp_pool.tile([128, 512], F32, name="vp", tag="sc")
                for kc in range(KC):
                    nc.tensor.matmul(
                        vp[:, 0:KV],
                        xt_all[:, kc, vt * 128 : (vt + 1) * 128],
                        wv_all[:, kc, :],
                        start=(kc == 0),
                        stop=(kc == KC - 1),
                    )
                v_t = qkv.tile([128, NG, HD + 1], BF16, name=f"v{vt}", tag=f"v{vt}")
                nc.vector.tensor_copy(v_t[:, :, HD], vcol_sb[:, vt, :])
                nc.scalar.copy(
                    v_t[:, :, 0:HD],
                    vp[:, 0:KV].rearrange("p (g d) -> p g d", g=NG),
                )
                v_sb[vt] = v_t

            # s2=0 prologue inline: enough for every qt=0 unit. The s2=1
            # half is deferred into early unit steps (KV2_STEPS) so its PE
            # work fills the qt=0 pipeline-fill idle.
            for mt in range(2):
                k_half(0, mt)
            for vt in range(3):
                v_half(vt)
            if not DEFER_S2 or not DEFER_K:
                for mt in range(2):
                    k_half(1, mt)
            if not DEFER_S2:
                for vt in range(3, 6):
                    v_half(vt)

            _dump('kT0', kT_sb[0][:])
            _dump('v2', v_sb[2][:].rearrange("p g d -> p (g d)"))
            yn_sb = [
                [
                    ynp.tile([128, 128], BF16, name=f"yn{m}_{qt}", tag=f"yn{m}_{qt}")
                    for qt in range(NQT)
                ]
                for m in range(KC)
            ]
            qT_sb = [None] * KC

            # ---- software-pipelined attention over 64 (head, qtile) units --
            # Per unit u = (h, qt): scores (3 PE matmuls) land in half of a
            # paired PSUM tile; one Exp per head pair; per-unit band-mask
            # multiplies alternate DVE / Pool; av matmuls run at unit-lag-2;
            # the pair tail (recip + 2 fused-normalize copies + transpose)
            # runs one pair later so it never blocks ScalarE/DVE queues.
            state = {}  # pair index -> (sc, ex, y)

            def front_scores(u):
                h, qt = u
                hh, g, m = h % 2, h // 4, h // 2
                if MERGED:
                    # sc/ex layout [128 k, 3 r, 2 hh, 128 q]: one matmul per
                    # k-tile covers BOTH heads of the pair (rhs free = 256).
                    if hh == 0:
                        sc = scp_pool.tile([128, 3, 2, 128], F32, name="sc", tag="sc")
                        ex = expp.tile([128, 3, 2, 128], BF16, name="ex", tag="ex")
                        state[('p', m, qt)] = (sc, ex)
                        qT = qT_sb[m]
                        for r in range(3):
                            nc.tensor.matmul(
                                sc[:, r, :, :],
                                kT_sb[g][0:64, (qt + r) * 128 : (qt + r + 1) * 128],
                                qT[0:64, :, qt * 128 : (qt + 1) * 128],
                                start=True,
                                stop=True,
                            )
                        return
                    sc, ex = state[('p', m, qt)]
                    nc.scalar.activation(
                        ex[:], sc[:], mybir.ActivationFunctionType.Exp,
                        scale=EXPSCALE if FP8QK else 1.0,
                    )
                    nc.vector.tensor_tensor(
                        ex[:, 0:3:2, 0, :], ex[:, 0:3:2, 0, :], mask_sb[:], mybir.AluOpType.mult
                    )
                    MASK_ENG(nc).tensor_tensor(
                        ex[:, 0:3:2, 1, :], ex[:, 0:3:2, 1, :], mask_sb[:], mybir.AluOpType.mult
                    )
                    return
                if hh == 0:
                    sc = scp_pool.tile([128, 2, 512], F32, name="sc", tag="sc")
                    ex = expp.tile([128, 2, 3, 128], BF16, name="ex", tag="ex")
                    state[('p', m, qt)] = (sc, ex)
                sc, ex = state[('p', m, qt)]
                qT = qT_sb[m]
                for r in range(3):
                    if SCORES_FP8:
                        # DoubleRow with a zero second stationary lane: the
                        # second moving lane (next 128 columns, finite)
                        # contributes 0, so this is the plain 64-deep score
                        # matmul at 2x rate
                        nc.tensor.matmul(
                            sc[:, hh, r * 128 : (r + 1) * 128],
                            kT_sb[g][
                                hh * 64 : hh * 64 + 64, :,
                                (qt + r) * 128 : (qt + r + 1) * 128,
                            ],
                            qT[hh * 64 : hh * 64 + 64, qt * 128 : (qt + 2) * 128]
                            .rearrange("p (two n) -> p two n", two=2),
                            start=True,
                            stop=True,
                            perf_mode=mybir.MatmulPerfMode.DoubleRow,
                        )
                        continue
                    nc.tensor.matmul(
                        sc[:, hh, r * 128 : (r + 1) * 128],
                        kT_sb[g][hh * 64 : hh * 64 + 64, (qt + r) * 128 : (qt + r + 1) * 128],
                        qT[hh * 64 : hh * 64 + 64, qt * 128 : (qt + 1) * 128],
                        start=True,
                        stop=True,
                    )
                if hh == 1:
                    _esc = EXPSCALE if FP8QK else 1.0
                    if PAIRED_EXP:
                        nc.scalar.activation(
                            ex[:], sc[:, :, 0:384], mybir.ActivationFunctionType.Exp,
                            scale=_esc,
                        )
                    else:
                        nc.scalar.activation(
                            ex[:, 0], sc[:, 0, 0:384], mybir.ActivationFunctionType.Exp,
                            scale=_esc,
                        )
                        nc.scalar.activation(
                            ex[:, 1], sc[:, 1, 0:384], mybir.ActivationFunctionType.Exp,
                            scale=_esc,
                        )
                    # band masks: only the two triangular k-tiles (the middle
                    # tile is all-valid; halo tokens are zeroed via the v
                    # ones-column), split across DVE and Pool
                    nc.vector.tensor_tensor(
                        ex[:, 0, 0:3:2, :], ex[:, 0, 0:3:2, :], mask_sb[:], mybir.AluOpType.mult
                    )
                    MASK_ENG(nc).tensor_tensor(
                        ex[:, 1, 0:3:2, :], ex[:, 1, 0:3:2, :], mask_sb[:], mybir.AluOpType.mult
                    )
                    if m == 0 and qt == 0:
                        _dump('ex00', ex[:].rearrange("p a r c -> p a (r c)"))

            def av_block(u):
                h, qt = u
                hh, g, m = h % 2, h // 4, h // 2
                sc, ex = state[('p', m, qt)]
                if hh == 0:
                    yg = yp_pool.tile([128, 2, HD + 1], F32, name="y4", tag="y4")
                    state[('g', m, qt)] = yg
                yg = state[('g', m, qt)]
                # middle tile (r=1) first: it carries no band mask, so this
                # matmul only waits on the exp — the masked outer tiles
                # accumulate afterwards, hiding one mask latency in the drain
                order = (1, 0, 2) if AV_MID_FIRST else (0, 1, 2)
                for j, r in enumerate(order):
                    nc.tensor.matmul(
                        yg[:, hh, :],
                        ex[:, r, hh, :] if MERGED else ex[:, hh, r, :],
                        v_sb[qt + r][:, g, :],
                        start=(j == 0),
                        stop=(j == 2),
                    )
                if m == 0 and qt == 0 and hh == 1:
                    _dump('y00', yg[:].rearrange("p a d -> p (a d)"))

            drain_flag = [False]

            def tail1(p):
                m, qt = p
                state.pop(('p', m, qt))
                yg = state.pop(('g', m, qt))
                ytn = ytnp.tile([128, 2, HD], BF16, name="ytn", tag="ytn")
                if NORM_DIV:
                    # one fused divide: numerator columns over the ones-column
                    # denominator, no separate reciprocal
                    nc.vector.tensor_tensor(
                        ytn[:],
                        yg[:, :, 0:HD],
                        yg[:, :, HD : HD + 1].broadcast_to([128, 2, HD]),
                        mybir.AluOpType.divide,
                    )
                    state[('t', m, qt)] = ytn
                    return
                rb = rrp.tile([128, 2], F32, name="rb", tag="rb")
                nc.vector.reciprocal(rb[:], yg[:, :, HD])
                if NORM_BCAST:
                    nc.vector.tensor_tensor(
                        ytn[:],
                        yg[:, :, 0:HD],
                        rb[:].unsqueeze(2).broadcast_to([128, 2, HD]),
                        mybir.AluOpType.mult,
                    )
                else:
                    nc.vector.tensor_scalar_mul(ytn[:, 0, :], yg[:, 0, 0:HD], rb[:, 0:1])
                    nc.vector.tensor_scalar_mul(ytn[:, 1, :], yg[:, 1, 0:HD], rb[:, 1:2])
                state[('t', m, qt)] = ytn
                if m == 0 and qt == 0:
                    _dump('ytn00', ytn[:].rearrange("p a d -> p (a d)"))

            def tail2(p):
                m, qt = p
                ytn = state.pop(('t', m, qt))
                if qt < DMA_TP_QT and not drain_flag[0]:
                    # XBAR transpose straight to SBUF: off PE, off DVE, no
                    # PSUM. ~2.3us latency absorbed by the out-proj slack.
                    nc.sync.dma_start_transpose(yn_sb[m][qt][:], ytn[:])
                    if m == 0 and qt == 0:
                        _dump('yn00', yn_sb[m][qt][:])
                    return
                # PE transpose via identity + DVE copy back to SBUF; during
                # the drain the score pool is idle — use it so pj stays free
                # for the final output projections
                if drain_flag[0]:
                    tp = scp_pool.tile([128, 128], BF16, name="tp", tag="sc")
                else:
                    tp = TP_POOL[0].tile([128, 128], BF16, name="tp", tag=TP_POOL[1])
                nc.tensor.matmul(
                    tp[:], ytn[:], id_sb[:], start=True, stop=True, is_transpose=True
                )
                yn_eng = nc.vector if YN_ENG == "vector" else nc.gpsimd
                yn_eng.tensor_copy(yn_sb[m][qt][:], tp[:])
                if m == 0 and qt == 0:
                    _dump('yn00', yn_sb[m][qt][:])

            def q_proj(m):
                qp = pj.tile([128, 512], F32, name="qp", tag="pj")
                if FP8QK:
                    for kcp in range(4):
                        for hl in range(2):
                            nc.tensor.matmul(
                                qp[:],
                                wq_sb[m // 2][:, m % 2, hl, kcp, :, :],
                                xt8_sb[:, kcp, :, WINDOW:TE],
                                start=(hl == 0 and kcp == 0),
                                stop=(hl == 1 and kcp == 3),
                                perf_mode=mybir.MatmulPerfMode.DoubleRow,
                            )
                else:
                    for kc in range(KC):
                        nc.tensor.matmul(
                            qp[:],
                            wq_sb[m // 2][:, m % 2, kc, :],
                            xt_all[:, kc, WINDOW:TE],
                            start=(kc == 0),
                            stop=(kc == KC - 1),
                        )
                if MERGED:
                    # repack to [64 hd, 2 hh, TQ]: hh=1 shifts partitions -64
                    qT = qkv.tile([64, 2, TQ], BF16, name=f"qT{m}", tag=f"qT{m}")
                    nc.vector.tensor_scalar_add(
                        qT[:, 0, :], qp[0:64, :], bq_all[:, 0, m : m + 1]
                    )
                    eng = nc.vector if QT_HH1_ENG == "vector" else nc.scalar
                    if eng is nc.vector:
                        eng.tensor_scalar_add(
                            qT[:, 1, :], qp[64:128, :], bq_all[:, 1, m : m + 1]
                        )
                    else:
                        eng.add(qT[:, 1, :], qp[64:128, :], bq_all[:, 1, m : m + 1])
                    qT_sb[m] = qT
                    return
                if SCORES_FP8:
                    # +128 finite pad columns so the DoubleRow moving pair
                    # (columns [span, span+128]) never reads junk at qt=3;
                    # the pad meets the zero stationary lane, so any finite
                    # value is fine
                    qT = qkv.tile([128, TQ + 128], F8, name=f"qT{m}", tag=f"qT{m}")
                    nc.gpsimd.memset(qT[:, TQ : TQ + 128], 0.0)
                    # 0.25*(64 q) + 16*bq -> 16*(q+bq) in fp8
                    nc.vector.tensor_scalar(
                        qT[:, 0:TQ], qp[:], 0.25, bq_all[:, m : m + 1],
                        mybir.AluOpType.mult, mybir.AluOpType.add,
                    )
                    qT_sb[m] = qT
                    return
                qT = qkv.tile([128, TQ], BF16, name=f"qT{m}", tag=f"qT{m}")
                nc.vector.tensor_scalar_add(qT[:], qp[:], bq_all[:, m : m + 1])
                qT_sb[m] = qT
                if m == 0:
                    _dump('qT0', qT[:])

            def out_proj_mm(tt, n2, op, m_lo, m_hi):
                for m in range(m_lo, m_hi):
                    nc.tensor.matmul(
                        op[:],
                        yn_sb[m][tt][:],
                        wo_all[:, m, n2 * 512 : (n2 + 1) * 512],
                        start=(m == 0),
                        stop=(m == KC - 1),
                    )

            def out_proj_fin(tt, n2, ob, op, fast=False, q2=None):
                if fast:
                    # tail: split the copy across ScalarE and DVE and DMA each
                    # half as soon as it lands (second DMA optionally on
                    # another hwdge queue so the gens overlap)
                    nc.scalar.copy(ob[:, n2 * 512 : n2 * 512 + 256], op[:, 0:256])
                    nc.vector.tensor_copy(
                        ob[:, n2 * 512 + 256 : (n2 + 1) * 512], op[:, 256:512]
                    )
                    nc.sync.dma_start(
                        out=out[tt * 128 : (tt + 1) * 128, n2 * 512 : n2 * 512 + 256],
                        in_=ob[:, n2 * 512 : n2 * 512 + 256],
                    )
                    (q2 or nc.sync).dma_start(
                        out=out[tt * 128 : (tt + 1) * 128, n2 * 512 + 256 : (n2 + 1) * 512],
                        in_=ob[:, n2 * 512 + 256 : (n2 + 1) * 512],
                    )
                    return
                if tt < 3 and FIN_ENG == "gpsimd":
                    # mid-stream fins off Act so they never stall the exp
                    # cadence; Pool has ample headroom
                    nc.gpsimd.tensor_copy(ob[:, n2 * 512 : (n2 + 1) * 512], op[:])
                elif tt < 3 and FIN_ENG == "vector":
                    nc.vector.tensor_copy(ob[:, n2 * 512 : (n2 + 1) * 512], op[:])
                elif tt == 3 and n2 == 1 and FIN2_DVE:
                    # overlap the last two fin copies (Act would serialize)
                    nc.vector.tensor_copy(ob[:, n2 * 512 : (n2 + 1) * 512], op[:])
                else:
                    nc.scalar.copy(ob[:, n2 * 512 : (n2 + 1) * 512], op[:])
                (q2 or nc.sync).dma_start(
                    out=out[tt * 128 : (tt + 1) * 128, n2 * 512 : (n2 + 1) * 512],
                    in_=ob[:, n2 * 512 : (n2 + 1) * 512],
                )

            def out_proj_half(tt, n2, ob):
                op = pj.tile([128, 512], F32, name="op", tag="pj")
                out_proj_mm(tt, n2, op, 0, KC)
                out_proj_fin(tt, n2, ob, op, fast=FIN_FAST)

            # step schedule: unit stream with lagged av / pair tails, plus
            # q projections (qt 0) and output projections (qt >= 1) as PE
            # filler inserted at fixed pair positions
            units = [(2 * m + hh, qt) for qt in range(NQT) for m in range(KC) for hh in range(2)]
            ob_tiles = {}
            pre = {}  # step index -> list of callables
            for m in range(KC):
                pre.setdefault(Q_STEP * m, []).append(lambda m=m: q_proj(m))
            late_ops = []
            for tt in range(NQT):
                # first step at which every yn[m][tt] write has been emitted:
                # last pair's tail2 of phase tt runs at step (tt*KC+7)*2+1+TAIL2_LAG
                s0 = (tt * KC + KC - 1) * 2 + 1 + TAIL2_LAG + 1 + OP_SLACK

                def _op(tt=tt):
                    ob = outp.tile([128, C], BF16 if OUT_BF16 else F32, name="ob", tag="ob")
                    ob_tiles[tt] = ob
                    out_proj_half(tt, 0, ob)

                def _op2(tt=tt):
                    out_proj_half(tt, 1, ob_tiles[tt])

                if s0 + OP_GAP < 2 * NQT * KC:
                    pre.setdefault(s0, []).append(_op)
                    pre.setdefault(s0 + OP_GAP, []).append(_op2)
                else:
                    late_ops.append((_op, _op2))

            if DEFER_S2:
                if DEFER_K:
                    pre.setdefault(KV2_STEPS[0], []).append(lambda: k_half(1, 0))
                    pre.setdefault(KV2_STEPS[1], []).append(lambda: k_half(1, 1))
                pre.setdefault(KV2_STEPS[2], []).append(lambda: v_half(3, True))
                pre.setdefault(KV2_STEPS[3], []).append(lambda: v_half(4, True))
                pre.setdefault(KV2_STEPS[4], []).append(lambda: v_half(5, True))
            n = len(units)
            for s in range(n):
                for fn in pre.get(s, []):
                    fn()
                front_scores(units[s])
                if 0 <= s - AV_LAG:
                    av_block(units[s - AV_LAG])
                if 0 <= s - TAIL1_LAG and units[s - TAIL1_LAG][0] % 2 == 1:
                    h, qt = units[s - TAIL1_LAG]
                    tail1((h // 2, qt))
                if 0 <= s - TAIL2_LAG and units[s - TAIL2_LAG][0] % 2 == 1:
                    h, qt = units[s - TAIL2_LAG]
                    tail2((h // 2, qt))
            # fast drain: finish remaining av/tail chains in dependency order;
            # late output projections start accumulating as soon as the yn
            # tiles they need exist, with only the last head-pairs deferred
            drain_flag[0] = True
            if 2 not in ob_tiles:
                ob2 = outp.tile([128, C], BF16 if OUT_BF16 else F32, name="ob", tag="ob")
                ob_tiles[2] = ob2
                out_proj_half(2, 0, ob2)
            op3 = {}
            ob3 = None
            for idx in range(n - TAIL2_LAG, n):
                u = units[idx]
                if idx >= n - AV_LAG:
                    av_block(u)
                if u[0] % 2 == 1:
                    p = (u[0] // 2, u[1])
                    if idx >= n - TAIL1_LAG:
                        tail1(p)
                    tail2(p)
                if idx == n - TAIL2_LAG + OP2B_OFF:
                    out_proj_half(2, 1, ob_tiles[2])
                if idx == n - OP3_OFF:
                    # yn[0..5][3] all written: start the final projections
                    ob3 = outp.tile([128, C], BF16 if OUT_BF16 else F32, name="ob", tag="ob")
                    for n2 in range(2):
                        op3[n2] = pj.tile([128, 512], F32, name="op", tag="pj")
                        out_proj_mm(3, n2, op3[n2], 0, KC - 2)
            for n2 in range(2):
                out_proj_mm(3, n2, op3[n2], KC - 2, KC)
                out_proj_fin(
                    3, n2, ob3, op3[n2], fast=FIN_FAST_LAST,
                    q2=(nc.scalar if (LAST_DMA_SCALAR and n2 == 1) else None),
                )


    _split_multi_waits(nc)
    return nc


_NC = None


def _get_nc():
    global _NC
    if _NC is None:
        _NC = _build_program()
    return _NC


def _host_prep(x, Wq, bq, Wk, bk, Wv, bv, Wo, bo):
    x = np.ascontiguousarray(np.asarray(x, dtype=np.float32))
    Wq = np.asarray(Wq, np.float32)
    bq = np.asarray(bq, np.float32)
    Wk = np.asarray(Wk, np.float32)
    bk = np.asarray(bk, np.float32)
    Wv = np.asarray(Wv, np.float32)
    bv = np.asarray(bv, np.float32)
    Wo = np.asarray(Wo, np.float32)
    bo = np.asarray(bo, np.float32)

    scale = np.float32(1.0 / np.sqrt(HD))

    def _split8(w, s):
        # w*s as fp8 hi + lo residual (both exactly representable in e4m3)
        ws = np.asarray(w, np.float32) * np.float32(s)
        hi = np.clip(ws, -240, 240).astype(NPF8).astype(np.float32)
        lo = (ws - hi).astype(NPF8).astype(np.float32)
        return np.stack([hi, lo])  # [2, *w.shape]

    if FP8QK:
        # Wq carries xWS; the 1/sqrt(hd) and 1/WS^2 descale fold into the
        # device-side exp scale. hi/lo split pairs ride the DoubleRow kc pairs.
        # wq8[mp, p, m2, hl, kcp, i, b] = Whl[(2*kcp+i)*128+p, (2*mp+m2)*128+b]
        arr = _split8(Wq, WS).reshape(2, 4, 2, 128, 4, 2, 128)
        wq_t = np.ascontiguousarray(arr.transpose(4, 3, 5, 0, 1, 2, 6).astype(NPF8))
        arr = _split8(Wk, WS).reshape(2, 4, 2, 128, 2, 128)
        wk_h = np.ascontiguousarray(arr.transpose(3, 4, 0, 1, 2, 5).astype(NPF8))
        bqs = np.float32(16.0 if SCORES_FP8 else WS)
    else:
        # (m, kc)-tiled, pre-scaled Wq: wq_t[m, p, kc, b] = Wq[kc*128+p, m*128+b]
        wq_t = np.ascontiguousarray(
            (Wq * scale).reshape(KC, 128, KC, 128).transpose(2, 1, 0, 3).astype(NPBF16)
        )
        wk_h = np.ascontiguousarray(
            Wk.reshape(KC, 128, 2, 128).transpose(2, 1, 0, 3).astype(NPBF16)
        )
        bqs = scale
    if MERGED:
        # [64 hd-half, 2 hh, KC]: bq2[p, hh, m] = bq[m*128 + hh*64 + p]
        bq_h = np.ascontiguousarray((bq * bqs).reshape(KC, 2, 64).transpose(2, 1, 0))
    else:
        bq_h = np.ascontiguousarray((bq * bqs).reshape(C, 1))
    wv_h = np.ascontiguousarray(Wv.reshape(KC, 128, KV).transpose(1, 0, 2).astype(NPBF16))
    wo_h = np.ascontiguousarray(Wo.reshape(KC, 128, C).transpose(1, 0, 2).astype(NPBF16))
    bk_h = np.ascontiguousarray(
        ((bk * WS) if FP8QK else bk).reshape(1, KV).astype(NPBF16)
    )
    ident_h = np.ascontiguousarray(np.eye(128, dtype=np.float32).astype(NPBF16))

    # constant triangular band masks for the outer two k-tiles of each
    # q-tile strip (kt = qt: valid b >= a; kt = qt + 2: valid b <= a); the
    # middle tile is all-valid. Halo tokens are excluded via vcol below.
    b_idx = np.arange(128)[:, None]  # k index within tile
    a_idx = np.arange(128)[None, :]  # q index within tile
    mk = np.empty((128, 2, 128), np.float32)
    mk[:, 0, :] = (b_idx >= a_idx).astype(np.float32)
    mk[:, 1, :] = (b_idx <= a_idx).astype(np.float32)
    mask_h = np.ascontiguousarray(mk.astype(NPBF16))
    # per-core token-validity for the v ones-column (zero on the left halo)
    vcols = {}
    for c in range(4):
        t_glob = c * TQ - WINDOW + np.arange(TE)
        valid = (t_glob >= 0).astype(np.float32)
        vcols[c] = np.ascontiguousarray(
            np.repeat(valid.reshape(6, 128).T[:, :, None], NG, axis=2).astype(NPBF16)
        )

    in_maps = []
    for core in range(NCORES):
        bb, c = core // 4, core % 4
        t0 = c * TQ - WINDOW
        xe = np.zeros((TE, C), np.float32)
        lo = max(t0, 0)
        xe[lo - t0 : TE, :] = x[bb, lo : t0 + TE, :]
        im = {}
        if FP8QK:
            # xt8[p, kcp, i, t] = fp8(xe[t, (2*kcp+i)*128 + p])
            im["xt8"] = np.ascontiguousarray(
                np.clip(xe.T, -240, 240)
                .reshape(4, 2, 128, TE)
                .transpose(2, 0, 1, 3)
                .astype(NPF8)
            )
        in_maps.append(
            {
                **im,
                "xt": np.ascontiguousarray(
                    xe.T.reshape(KC, 128, TE).transpose(1, 0, 2).astype(NPBF16)
                ),
                "wq": wq_t,
                "wk": wk_h,
                "wv": wv_h,
                "wo": wo_h,
                "bq": bq_h,
                "bk": bk_h,
                "maskp": mask_h,
                "vcol": vcols[c],
                "ident": ident_h,
            }
        )

    # exact linear bias correction applied host-side:
    # y = att@(v+bv) = att@v + bv (softmax rows sum to 1), so
    # out += bv_rep @ Wo + bo
    bv_rep = np.concatenate([bv[(h // NG) * HD : (h // NG + 1) * HD] for h in range(NH)])
    corr = bv_rep.astype(np.float64) @ Wo.astype(np.float64) + bo.astype(np.float64)
    return in_maps, corr.astype(np.float32)


LAST_RESULTS = None


def kernel(x, Wq, bq, Wk, bk, Wv, bv, Wo, bo):
    global LAST_RESULTS
    in_maps, corr = _host_prep(x, Wq, bq, Wk, bk, Wv, bv, Wo, bo)
    nc = _get_nc()
    res = run_bass_kernel_spmd(nc, in_maps, core_ids=list(range(NCORES)))
    LAST_RESULTS = res
    out = np.empty((B, T, C), np.float32)
    for core in range(NCORES):
        bb, c = core // 4, core % 4
        out[bb, c * TQ : (c + 1) * TQ, :] = np.asarray(
            res.results[core]["out"], dtype=np.float32
        )
    out += corr[None, None, :]
    return out

